# revision 38
# baseline (speedup 1.0000x reference)
"""Per-frame RMS energy (STFT framing: n_fft=1024, hop=256, center/reflect pad)
over a [16, 1048576] f32 signal -> [16, 4096, 1] f32.

Trainium2 Bass/Tile kernel, data-parallel over batch across 8 NeuronCores
(2 signals per core). Each 1024-sample frame is exactly 4 consecutive
256-sample hop blocks, so we compute per-block sums of squares (one read of
every input byte -> memory-bound optimal), then a sliding sum of 4 plus
sqrt(mean).

Layout: partition p of a signal owns frames p*32..p*32+31; its input row is
the naturally aligned x[p*8192 : (p+1)*8192]. ext[p, u] = s_pad[p*32+u]
(u in 0..34) where s_pad[b] is the padded-signal 256-block sum of squares;
cols 2..33 come straight from the block reduces, the 3-value seam from the
neighbor partition comes via two tiny PE shift-matmuls + PSUM->SBUF copies,
and the reflect-pad edge values are derived from existing block sums with
single-sample corrections (s_pad[1] = S[0] - x[0]^2 + x[256]^2 etc.), so no
extra edge loads or 256-wide edge reductions are needed.

Measured-window discipline: the profiler's exec window opens at the FIRST
non-overhead instruction and closes at the last instruction of the NRT
teardown (a fixed ~7.5us epilogue: an all-engine rendezvous on a runtime
semaphore, then each engine serially clears its ~51-sem share of the 253
user semaphores, then a final rendezvous - no kernel content can remove
it). Within the window the kernel (a) keeps the pre-stream ramp minimal:
the const-AP memsets Bass.__init__ emits are suppressed and the activation
bias zeros come from a gpsimd memset instead of a DMA, so only the ACT
table preloads and the shifts trigger precede the first bulk trigger; and
(b) minimizes the post-stream tail: sig0 streams first in coarse chunks,
sig1 last with progressively smaller chunks, sig1's (24,2) square runs on
gpsimd in parallel with ACT's final 1-block chunks, window-of-4 sums run on DVE right
behind the last reduce, and both signals share a single sqrt and a single
output-store DMA.

Engine plan:
 - Sync HWDGE ring: the 12 bulk chunk loads (sig0 coarse then sig1 fine),
   then the one combined output store. 2-8KB per-partition lines keep the
   16 DMA engines at their ~24.5 B/ns packet-rate limit (~390 GB/s
   aggregate, the per-core DMA bus roofline; chip-level HBM contention
   with the other 7 cores adds run-to-run variance). Tile round-robins
   DMA-completion sems over only 8 lanes, so chunk count and order are
   chosen so every lane reuse gates on a consumer that retired >5us before
   the reusing chunk's data turn (a stalled trigger starves the descriptor
   FIFOs and stretches the stream).
 - Scalar/ACT + its HWDGE ring: shifts-matrix load, act-table preload via
   a dummy sqrt (both ACT tables land in the stream ramp, not the tail),
   all squares except sig1's (24,2) (f32 in, bf16 out), one final [128,64]
   sqrt covering both signals.
 - Vector/DVE: block reduces - bf16 pairwise-add levels (DVE 2x mode,
   ~0.52 ns/out-elem) down to 64 values per block, then one grouped k=64
   f32 reduce (~1.19 ns/in-elem, its floor; reduces get no 2x/4x mode on
   real HW even with bf16 output) - plus seam PSUM->SBUF copies and the
   window-of-4 pairwise sums.
 - GpSimd: zeros-bias memset, the tiny edge-correction arithmetic, and
   sig1's (24,2) square (GPSIMD_IMPL_EFFICIENCY is only 0.42, so it gets
   exactly one small tail chunk).
 - PE: two tiny shift matmuls per signal for the cross-partition seam.
"""

import sys
import types

import numpy as np

import concourse.bacc as bacc
import concourse.bass as bass
import concourse.mybir as mybir
import concourse.tile as tile
from concourse.bass_utils import run_bass_kernel_spmd
from concourse.vector_clock import ScopedClock


def _install_ntff_hook_shim():
    """The image's antenv lacks axon_hooks; if a caller turns on tracing
    (e.g. via BASS_TRACE=1), run_bass_kernel_spmd imports it. Provide the
    ctypes-based hook so that path works instead of raising."""
    try:
        import antenv.axon_hooks  # noqa: F401

        return
    except ImportError:
        pass
    try:
        from trn_agent_boot.trn_boot import _ntff_profile_via_ctypes

        hook = _ntff_profile_via_ctypes("/opt/axon/libaxon_pjrt.so")
    except Exception:
        hook = None
    mod = types.ModuleType("antenv.axon_hooks")
    mod.get_axon_ntff_profile_hook = lambda: hook
    mod.set_axon_ntff_profile_hook = lambda h: None
    sys.modules["antenv.axon_hooks"] = mod


_install_ntff_hook_shim()




class SlimExitTileContext(tile.TileContext):
    """TileContext whose exit sequence drops the second all-engine barrier.

    The stock epilogue is drain -> barrier -> sem clear -> barrier. The
    first barrier guarantees every engine is idle before the gpsimd range
    sem-clear runs; the trailing barrier only re-synchronizes engines that
    are each about to run off the end of their own queues, so skipping it
    is safe (NRT completion still waits for every queue, and the sem state
    a re-execution needs is restored by the clear).
    """

    def _drain_and_barrier(self, tick_clock, wait_clock):
        # Single Pool-side rendezvous: gpsimd waits out the full vector clock
        # (all compute retired, all DMA receipts landed) and then resets sem
        # state. No all-engine barrier at all: every other engine's queue
        # simply ends after its last real instruction, so the per-engine
        # event-semaphore restore chains the toolchain appends run early,
        # overlapped with the stream, instead of serialized after a barrier.
        drain_inst = self.nc.gpsimd.drain()
        wait_clock.add_sem_waits(
            drain_inst.ins, ScopedClock({None: tick_clock.global_clock})
        )
        assert self.sems is not None
        popped = self.nc._tile_sem_poison_stack.pop()
        assert popped is self._sem_poison
        self.nc.clear_and_free_semaphores(list(self.sems.allocated().values()))


# Problem constants (self-contained; must match the grader's input spec)
B = 16                 # signals in the batch
T = 1048576            # samples per signal
N_FFT = 1024
HOP = 256
N_CORES = 8
SIG_PER_CORE = B // N_CORES   # 2
P = 128                       # SBUF partitions
NBLK = T // HOP               # 4096 hop blocks per signal
CPB = NBLK // P               # 32 output frames per partition
SPP = T // P                  # 8192 samples per partition row
NFRAMES = NBLK                # 4096 output frames per signal

# Per-signal chunks of the 8192-sample partition row, in 256-blocks
# (block_offset, n_blocks). The chunk holding the seam-source blocks 30,31
# goes first and the one holding block 0 second, so the two seam matmuls
# run early. sig0 (mid-stream, tail-irrelevant) uses coarse chunks to keep
# the total HWDGE DMA count low: Tile round-robins DMA-completion sems over
# only 8 lanes, and a trigger whose lane is still held by a chunk with
# unretired consumers stalls, starving the descriptor FIFOs mid-stream.
# With 12 bulk DMAs + shifts + store, every lane reuse gates on a consumer
# that retired >5us before the reusing chunk's data turn. sig1 keeps small
# final chunks to shorten the post-stream compute tail.
CHUNKS0 = [(28, 4), (0, 4), (4, 8), (12, 8), (20, 8)]
CHUNKS1 = [(28, 4), (0, 4), (4, 8), (12, 4), (16, 4), (20, 4), (24, 2), (26, 1), (27, 1)]
SIG_CHUNKS = [CHUNKS0, CHUNKS1]
# (sig, block_offset) squares that run on gpsimd, not ACT (tail-parallel)
GPSIMD_SQ = {(1, 24)}

F32 = mybir.dt.float32
BF16 = mybir.dt.bfloat16
AF = mybir.ActivationFunctionType
AX = mybir.AxisListType
ADD = mybir.AluOpType.add
SUB = mybir.AluOpType.subtract
MULT = mybir.AluOpType.mult


def _block_reduce(nc, casc_pool, ext, tsq, b0, nb, is_f32=False):
    """ext[:, 2+b0 : 2+b0+nb] = per-256-block sums of tsq (bf16 squares).

    Pairwise bf16 add levels (DVE 2x mode, ~0.52 ns/out-elem) down to 64
    values per block, then one grouped k=64 f32 reduce (~1.19 ns/in-elem).
    Cheaper than a single k=256 reduce for nb>=4; tiny chunks and the f32
    gpsimd-squared chunk keep the single reduce (per-op overhead isn't
    worth it).
    """
    out = ext[:, 2 + b0 : 2 + b0 + nb]
    if is_f32 or nb <= 2:
        nc.vector.tensor_reduce(
            out=out,
            in_=tsq[:, :].rearrange("p (g k) -> p g k", k=HOP),
            axis=AX.X,
            op=ADD,
        )
        return
    cur, k = tsq, HOP
    while k > 64:
        half = k // 2
        u = casc_pool.tile([P, nb * half], BF16, tag=f"u{k}_{nb}")
        rin = cur[:, :].rearrange("p (g two k) -> p g two k", two=2, k=half)
        rout = u[:, :].rearrange("p (g one k) -> p g one k", one=1, k=half)
        nc.vector.tensor_add(
            out=rout, in0=rin[:, :, 0:1, :], in1=rin[:, :, 1:2, :]
        )
        cur, k = u, half
    nc.vector.tensor_reduce(
        out=out,
        in_=cur[:, :].rearrange("p (g k) -> p g k", k=k),
        axis=AX.X,
        op=ADD,
    )


def build_bass():
    # Bacc (not plain Bass): its compile pipeline splits multi-sem waits into
    # event-semaphore instructions, which this walrus build requires.
    #
    # Bass.__init__ ends with const-AP memsets + an all-engine barrier. The
    # memsets are non-overhead instructions that would open the profiler's
    # exec window ~0.8us before the first load trigger, so suppress both
    # (this kernel reads no const APs: activations get an explicit
    # gpsimd-memset zeros tile that Tile orders itself).
    orig_barrier = bass.Bass.all_engine_barrier
    orig_memset = bass.BassGpSimd.memset
    bass.Bass.all_engine_barrier = lambda self, *, sem_only=False: None
    bass.BassGpSimd.memset = lambda self, ap, constant: None
    try:
        nc = bacc.Bacc()
    finally:
        bass.Bass.all_engine_barrier = orig_barrier
        bass.BassGpSimd.memset = orig_memset
    x = nc.dram_tensor("signal", [SIG_PER_CORE, T], F32, kind="ExternalInput")
    sh = nc.dram_tensor("shifts", [P, 2 * P], F32, kind="ExternalInput")
    y = nc.dram_tensor("out", [SIG_PER_CORE, NFRAMES], F32, kind="ExternalOutput")

    xr = x[:, :].rearrange("b (p f) -> b p f", p=P)   # [2, 128, 8192]
    yr = y[:, :].rearrange("b (p c) -> p b c", p=P)   # [128, 2, 32]

    with SlimExitTileContext(nc) as tc:
        with (
            tc.tile_pool(name="inp", bufs=2) as inp_pool,
            tc.tile_pool(name="sq", bufs=6) as sq_pool,
            tc.tile_pool(name="casc", bufs=2) as casc_pool,
            tc.tile_pool(name="ext", bufs=2) as ext_pool,
            tc.tile_pool(name="small", bufs=2) as small_pool,
            tc.tile_pool(name="ps", bufs=2, space=bass.MemorySpace.PSUM) as psum_pool,
        ):
            # Phase A: enqueue the WHOLE bulk stream on the sync ring first -
            # nothing else ever rides this ring until the output store, so
            # it is never head-of-line blocked. Keyed by block offset since
            # the two signals stream their chunks in different orders.
            tins = [dict() for _ in range(SIG_PER_CORE)]
            shm = small_pool.tile([P, 2 * P], F32, tag="shm")
            n_emitted = 0
            for sig in range(SIG_PER_CORE):
                for b0, nb in SIG_CHUNKS[sig]:
                    ln = nb * HOP
                    tin = inp_pool.tile([P, ln], F32, tag=f"tin{b0}_{nb}")
                    nc.sync.dma_start(
                        out=tin[:, :],
                        in_=xr[sig, :, b0 * HOP : b0 * HOP + ln],
                    )
                    tins[sig][b0] = tin
                    n_emitted += 1
                    if n_emitted == 1:
                        # Shift matrices for the PE seam matmuls (cols
                        # 0:128 = down-shift, 128:256 = up-shift), on the
                        # scalar ring; lands ~9.5us, first matmul needs it
                        # ~13us. Emitted here (2nd in DMA order) so its
                        # completion-sem lane is recycled by a late sig1
                        # chunk with no stall. Seam copies as DMAs are
                        # poison in every variant measured: SWDGE transfers
                        # slow concurrent HWDGE bulk ~40%, and sync-ring
                        # descriptors interleave into the bulk FIFO. The
                        # idle PE does them for free.
                        nc.scalar.dma_start(out=shm[:, :], in_=sh[:, :])

            # Activation-bias zeros via gpsimd memset (no DMA, no const
            # AP). Ready ~6.8us, first consumer needs it ~10.9us.
            zb = small_pool.tile([P, 1], F32, tag="zb")
            nc.gpsimd.memset(zb[:, :], 0.0)

            # Dummy Sqrt first: pulls both ACT table loads (2x 1.28us) into
            # the stream ramp instead of the critical tail. Reads zb so it
            # stays gated.
            dummy = small_pool.tile([1, 1], F32, tag="dummy")
            nc.scalar.activation(
                out=dummy[0:1, 0:1], in_=zb[0:1, 0:1], func=AF.Sqrt,
                bias=zb[0:1, 0:1],
            )

            exts = []
            scrs = []
            pss = []
            for sig in range(SIG_PER_CORE):
                ext = ext_pool.tile([P, 36], F32, tag="ext")
                scr = small_pool.tile([P, 8], F32, tag="scr")
                exts.append(ext)
                scrs.append(scr)

            # e2/ot: both signals' window sums / outputs side by side so one
            # sqrt and one store DMA cover everything.
            e2 = small_pool.tile([P, 2 * CPB], F32, tag="e2")
            ot = small_pool.tile([P, 2 * CPB], F32, tag="ot")

            # Phase B: per signal, per chunk: square (ACT, bf16 out; one
            # tail chunk of sig1 on gpsimd in f32), DVE block reduce; gpsimd
            # edge corrections and PE seam matmuls hang off the first two
            # chunks; seam copies + window sums on DVE after the last
            # reduce of each signal.
            for sig in range(SIG_PER_CORE):
                ext = exts[sig]
                scr = scrs[sig]
                # Compute-emission order: the gpsimd-reduced chunk LAST, so
                # its DVE k-reduce (gated on the gpsimd tree, ~32.5us) does
                # not head-of-line block the micro chunks' reduces (inputs
                # ready ~31.5us) in DVE's in-order queue. Stream order is
                # unchanged.
                order = [c for c in SIG_CHUNKS[sig] if (sig, c[0]) not in GPSIMD_SQ]
                order += [c for c in SIG_CHUNKS[sig] if (sig, c[0]) in GPSIMD_SQ]
                for b0, nb in order:
                    ln = nb * HOP
                    tin = tins[sig][b0]
                    if (sig, b0) in GPSIMD_SQ:
                        tsq = sq_pool.tile([P, ln], F32, tag=f"tsqg{b0}")
                        nc.gpsimd.tensor_mul(
                            out=tsq[:, :], in0=tin[:, :], in1=tin[:, :]
                        )
                        # gpsimd also runs the first 2 pairwise-add levels
                        # (256->64 per block, f32) itself: it is idle after
                        # this square while DVE's queue is the congested
                        # tail resource, so only a ~280ns k=64 reduce lands
                        # on DVE instead of a 679ns k=256 one - and the
                        # gpsimd handoff arrives ~1us sooner than a 3-level
                        # tree would.
                        cur, k = tsq, HOP
                        for _ in range(2):
                            half = k // 2
                            u = casc_pool.tile(
                                [P, nb * half], F32, tag=f"g{k}_{b0}"
                            )
                            rin = cur[:, :].rearrange(
                                "p (g two k) -> p g two k", two=2, k=half
                            )
                            rout = u[:, :].rearrange(
                                "p (g one k) -> p g one k", one=1, k=half
                            )
                            nc.gpsimd.tensor_add(
                                out=rout,
                                in0=rin[:, :, 0:1, :],
                                in1=rin[:, :, 1:2, :],
                            )
                            cur, k = u, half
                        nc.vector.tensor_reduce(
                            out=ext[:, 2 + b0 : 2 + b0 + nb],
                            in_=cur[:, :].rearrange("p (g k) -> p g k", k=k),
                            axis=AX.X,
                            op=ADD,
                        )
                    else:
                        tsq = sq_pool.tile([P, ln], BF16, tag="tsq")
                        nc.scalar.activation(
                            out=tsq[:, :], in_=tin[:, :], func=AF.Square,
                            bias=zb[:, 0:1],
                        )
                        _block_reduce(nc, casc_pool, ext, tsq, b0, nb)
                    if b0 == 28:
                        # Seam 1 matmul: psum[n, 0:2] = ext[n-1, 32:34]
                        # (blocks 30,31 from this chunk's reduce).
                        ps = psum_pool.tile([P, 4], F32, tag=f"ps{sig}")
                        pss.append(ps)
                        nc.tensor.matmul(
                            ps[:, 0:2], shm[:, 0:P], ext[:, 32:34],
                            start=True, stop=True,
                        )
                        # Right reflect edge by correction:
                        #   s_pad[4098] = sum x[T-257:T-1]^2
                        #              = S[4095] + x[T-257]^2 - x[T-1]^2
                        # S[4095] = ext[127, 33] (this chunk's reduce);
                        # x[T-257], x[T-1] are cols 767, 1023 of partition
                        # 127's (28,4) row. Compute partition bases must be
                        # 32-aligned, so run over the 96:128 quadrant; the
                        # garbage written to ext[96:127, 34] is overwritten
                        # by seam copy 2 (ordered after, same engine).
                        v2 = tin[96:128, 767:1024:256]           # [32, 2]
                        nc.gpsimd.tensor_mul(
                            out=scr[96:128, 0:2], in0=v2, in1=v2
                        )
                        nc.gpsimd.tensor_add(
                            out=scr[96:128, 2:3],
                            in0=ext[96:128, 33:34], in1=scr[96:128, 0:1],
                        )
                        nc.gpsimd.tensor_sub(
                            out=ext[96:128, 34:35],
                            in0=scr[96:128, 2:3], in1=scr[96:128, 1:2],
                        )
                    elif b0 == 0:
                        ps = pss[sig]
                        # Seam 2 matmul: psum[n, 2] = ext[n+1, 2] (block 0
                        # from this chunk's reduce). Runs on the idle PE.
                        nc.tensor.matmul(
                            ps[:, 2:3], shm[:, P : 2 * P], ext[:, 2:3],
                            start=True, stop=True,
                        )
                        # Seam copies PSUM -> ext on DVE, right here (mid-
                        # stream) so only the window sums remain in the
                        # post-stream tail. Row 0 of the down-shift and row
                        # 127 of the up-shift are zeros; the left
                        # corrections below overwrite ext[0, 0:2], and the
                        # split copy leaves ext[127, 34] (the right-edge
                        # value written at b0==28) intact.
                        nc.vector.tensor_copy(out=ext[:, 0:2], in_=ps[:, 0:2])
                        nc.vector.tensor_copy(
                            out=ext[0:96, 34:35], in_=ps[0:96, 2:3]
                        )
                        nc.vector.tensor_copy(
                            out=ext[96:127, 34:35], in_=ps[96:127, 2:3]
                        )
                        # Left reflect edges by correction (gpsimd; after
                        # copy1 so they overwrite the zero rows it writes at
                        # partition 0):
                        #   s_pad[1] = S[0] - x[0]^2 + x[256]^2
                        #   s_pad[0] = S[1] - x[256]^2 + x[512]^2
                        # S[0], S[1] = ext[0, 2:4]; x[0], x[256], x[512] are
                        # cols 0,256,512 of partition 0's (0,4) row.
                        v3 = tin[0:1, 0:513:256]                 # [1, 3]
                        nc.gpsimd.tensor_mul(
                            out=scr[0:1, 4:7], in0=v3, in1=v3
                        )
                        nc.gpsimd.tensor_sub(
                            out=scr[0:1, 2:4],
                            in0=ext[0:1, 2:4], in1=scr[0:1, 4:6],
                        )
                        nc.gpsimd.tensor_add(
                            out=ext[0:1, 1:2],
                            in0=scr[0:1, 2:3], in1=scr[0:1, 5:6],
                        )
                        nc.gpsimd.tensor_add(
                            out=ext[0:1, 0:1],
                            in0=scr[0:1, 3:4], in1=scr[0:1, 6:7],
                        )

                # Window-of-4 sums on DVE, right behind this signal's last
                # reduce (sig0's run in the gap before sig1's data-gated
                # reduces; sig1's are the critical tail, with no cross-
                # engine handoff before the final sqrt).
                # E[p, c] = ext[p, c] + ... + ext[p, c+3], via pairwise sums:
                # P1[c] = ext[c] + ext[c+1]; E[c] = P1[c] + P1[c+2].
                p1 = small_pool.tile([P, 34], F32, tag="p1")
                nc.vector.tensor_add(out=p1[:, :], in0=ext[:, 0:34], in1=ext[:, 1:35])
                nc.vector.tensor_add(
                    out=e2[:, sig * CPB : (sig + 1) * CPB],
                    in0=p1[:, 0:32], in1=p1[:, 2:34],
                )

            # One sqrt for both signals, one store DMA for both outputs.
            nc.scalar.activation(
                out=ot[:, :], in_=e2[:, :], func=AF.Sqrt,
                scale=1.0 / N_FFT, bias=zb[:, 0:1],
            )
            nc.sync.dma_start(
                out=yr,
                in_=ot[:, :].rearrange("p (b c) -> p b c", b=SIG_PER_CORE),
            )

    nc.finalize()
    return nc


_NC = None


def _make_shifts() -> np.ndarray:
    m = np.zeros((P, 2 * P), dtype=np.float32)
    # down-shift: psum[n] = mov[n-1]  ->  lhsT[k, n] = 1 iff n == k+1
    m[np.arange(P - 1), np.arange(1, P)] = 1.0
    # up-shift: psum[n] = mov[n+1]   ->  lhsT[k, n] = 1 iff n == k-1
    m[np.arange(1, P), P + np.arange(P - 1)] = 1.0
    return m


_SHIFTS = _make_shifts()


def run(signal: np.ndarray, trace: bool = False):
    global _NC
    sig = np.ascontiguousarray(np.asarray(signal, dtype=np.float32))
    assert sig.shape == (B, T), sig.shape
    if _NC is None:
        _NC = build_bass()
    in_maps = [
        {
            "signal": np.ascontiguousarray(
                sig[k * SIG_PER_CORE : (k + 1) * SIG_PER_CORE]
            ),
            "shifts": _SHIFTS,
        }
        for k in range(N_CORES)
    ]
    try:
        res = run_bass_kernel_spmd(
            _NC, in_maps, core_ids=list(range(N_CORES)), trace=trace
        )
    except Exception:
        # One retry: the shared trn2 devices occasionally surface a
        # transient NRT_EXEC_UNIT_UNRECOVERABLE from a prior session's
        # state; a fresh dispatch typically succeeds.
        res = run_bass_kernel_spmd(
            _NC, in_maps, core_ids=list(range(N_CORES)), trace=trace
        )
    out = np.concatenate([r["out"] for r in res.results], axis=0)
    return out.reshape(B, NFRAMES, 1).astype(np.float32), res


def kernel(signal: np.ndarray) -> np.ndarray:
    out, _ = run(signal, trace=False)
    return out


# revision 39
# speedup vs baseline: 1.0128x; 1.0128x over previous
"""Per-frame RMS energy (STFT framing: n_fft=1024, hop=256, center/reflect pad)
over a [16, 1048576] f32 signal -> [16, 4096, 1] f32.

Trainium2 Bass/Tile kernel, data-parallel over batch across 8 NeuronCores
(2 signals per core). Each 1024-sample frame is exactly 4 consecutive
256-sample hop blocks, so we compute per-block sums of squares (one read of
every input byte -> memory-bound optimal), then a sliding sum of 4 plus
sqrt(mean).

Layout: partition p of a signal owns frames p*32..p*32+31; its input row is
the naturally aligned x[p*8192 : (p+1)*8192]. ext[p, u] = s_pad[p*32+u]
(u in 0..34) where s_pad[b] is the padded-signal 256-block sum of squares;
cols 2..33 come straight from the block reduces, the 3-value seam from the
neighbor partition comes via two tiny PE shift-matmuls + PSUM->SBUF copies,
and the reflect-pad edge values are derived from existing block sums with
single-sample corrections (s_pad[1] = S[0] - x[0]^2 + x[256]^2 etc.), so no
extra edge loads or 256-wide edge reductions are needed.

Measured-window discipline: the profiler's exec window opens at the FIRST
non-overhead instruction and closes at the last instruction of the NRT
teardown (a fixed ~7.5us epilogue: an all-engine rendezvous on a runtime
semaphore, then each engine serially clears its ~51-sem share of the 253
user semaphores, then a final rendezvous - no kernel content can remove
it). Within the window the kernel (a) keeps the pre-stream ramp minimal:
the const-AP memsets Bass.__init__ emits are suppressed and the activation
bias zeros come from a gpsimd memset instead of a DMA, so only the ACT
table preloads and the shifts trigger precede the first bulk trigger; and
(b) minimizes the post-stream tail: sig0 streams first in coarse chunks,
sig1 last with progressively smaller chunks, sig1's (24,2) square runs on
gpsimd in parallel with ACT's final 1-block chunks, window-of-4 sums run on DVE right
behind the last reduce, and both signals share a single sqrt and a single
output-store DMA.

Engine plan:
 - Sync HWDGE ring: the 14 bulk chunk loads (sig0 coarse then sig1 fine),
   then the one combined output store. 2-8KB per-partition lines keep the
   16 DMA engines at their ~24.5 B/ns packet-rate limit (~390 GB/s
   aggregate, the per-core DMA bus roofline; chip-level HBM contention
   with the other 7 cores adds run-to-run variance). Tile round-robins
   DMA-completion sems over only 8 lanes, so chunk count and order are
   chosen so every lane reuse gates on a consumer that retired >5us before
   the reusing chunk's data turn (a stalled trigger starves the descriptor
   FIFOs and stretches the stream).
 - Scalar/ACT + its HWDGE ring: shifts-matrix load, act-table preload via
   a dummy sqrt (both ACT tables land in the stream ramp, not the tail),
   all squares except sig1's (24,2) (f32 in, bf16 out), one final [128,64]
   sqrt covering both signals.
 - Vector/DVE: block reduces - bf16 pairwise-add levels (DVE 2x mode,
   ~0.52 ns/out-elem) down to 64 values per block, then one grouped k=64
   f32 reduce (~1.19 ns/in-elem, its floor; reduces get no 2x/4x mode on
   real HW even with bf16 output) - plus seam PSUM->SBUF copies and the
   window-of-4 pairwise sums.
 - GpSimd: zeros-bias memset, the tiny edge-correction arithmetic, and
   sig1's (24,2) square plus its first two pairwise-add levels
   (GPSIMD_IMPL_EFFICIENCY is only 0.42, so it gets exactly one small
   tail chunk; handing DVE a k=64 reduce instead of k=256 keeps the
   saturated DVE tail short).
 - PE: two tiny shift matmuls per signal for the cross-partition seam.
"""

import sys
import types

import numpy as np

import concourse.bacc as bacc
import concourse.bass as bass
import concourse.mybir as mybir
import concourse.tile as tile
from concourse.bass_utils import run_bass_kernel_spmd
from concourse.vector_clock import ScopedClock


def _install_ntff_hook_shim():
    """The image's antenv lacks axon_hooks; if a caller turns on tracing
    (e.g. via BASS_TRACE=1), run_bass_kernel_spmd imports it. Provide the
    ctypes-based hook so that path works instead of raising."""
    try:
        import antenv.axon_hooks  # noqa: F401

        return
    except ImportError:
        pass
    try:
        from trn_agent_boot.trn_boot import _ntff_profile_via_ctypes

        hook = _ntff_profile_via_ctypes("/opt/axon/libaxon_pjrt.so")
    except Exception:
        hook = None
    mod = types.ModuleType("antenv.axon_hooks")
    mod.get_axon_ntff_profile_hook = lambda: hook
    mod.set_axon_ntff_profile_hook = lambda h: None
    sys.modules["antenv.axon_hooks"] = mod


_install_ntff_hook_shim()




class SlimExitTileContext(tile.TileContext):
    """TileContext whose exit sequence drops the second all-engine barrier.

    The stock epilogue is drain -> barrier -> sem clear -> barrier. The
    first barrier guarantees every engine is idle before the gpsimd range
    sem-clear runs; the trailing barrier only re-synchronizes engines that
    are each about to run off the end of their own queues, so skipping it
    is safe (NRT completion still waits for every queue, and the sem state
    a re-execution needs is restored by the clear).
    """

    def _drain_and_barrier(self, tick_clock, wait_clock):
        # Single Pool-side rendezvous: gpsimd waits out the full vector clock
        # (all compute retired, all DMA receipts landed) and then resets sem
        # state. No all-engine barrier at all: every other engine's queue
        # simply ends after its last real instruction, so the per-engine
        # event-semaphore restore chains the toolchain appends run early,
        # overlapped with the stream, instead of serialized after a barrier.
        drain_inst = self.nc.gpsimd.drain()
        wait_clock.add_sem_waits(
            drain_inst.ins, ScopedClock({None: tick_clock.global_clock})
        )
        assert self.sems is not None
        popped = self.nc._tile_sem_poison_stack.pop()
        assert popped is self._sem_poison
        self.nc.clear_and_free_semaphores(list(self.sems.allocated().values()))


# Problem constants (self-contained; must match the grader's input spec)
B = 16                 # signals in the batch
T = 1048576            # samples per signal
N_FFT = 1024
HOP = 256
N_CORES = 8
SIG_PER_CORE = B // N_CORES   # 2
P = 128                       # SBUF partitions
NBLK = T // HOP               # 4096 hop blocks per signal
CPB = NBLK // P               # 32 output frames per partition
SPP = T // P                  # 8192 samples per partition row
NFRAMES = NBLK                # 4096 output frames per signal

# Per-signal chunks of the 8192-sample partition row, in 256-blocks
# (block_offset, n_blocks). The chunk holding the seam-source blocks 30,31
# goes first and the one holding block 0 second, so the two seam matmuls
# run early. sig0 (mid-stream, tail-irrelevant) uses coarse chunks to keep
# the total HWDGE DMA count low: Tile round-robins DMA-completion sems over
# only 8 lanes, and a trigger whose lane is still held by a chunk with
# unretired consumers stalls, starving the descriptor FIFOs mid-stream.
# With 14 bulk DMAs + shifts + store, every lane reuse gates on a consumer
# that retired >5us before the reusing chunk's data turn. sig1 keeps small
# final chunks to shorten the post-stream compute tail.
CHUNKS0 = [(28, 4), (0, 4), (4, 8), (12, 8), (20, 8)]
CHUNKS1 = [(28, 4), (0, 4), (4, 8), (12, 4), (16, 4), (20, 4), (24, 2), (26, 1), (27, 1)]
SIG_CHUNKS = [CHUNKS0, CHUNKS1]
# (sig, block_offset) squares that run on gpsimd, not ACT (tail-parallel)
GPSIMD_SQ = {(1, 24)}

F32 = mybir.dt.float32
BF16 = mybir.dt.bfloat16
AF = mybir.ActivationFunctionType
AX = mybir.AxisListType
ADD = mybir.AluOpType.add
SUB = mybir.AluOpType.subtract
MULT = mybir.AluOpType.mult


def _block_reduce(nc, casc_pool, ext, tsq, b0, nb, is_f32=False):
    """ext[:, 2+b0 : 2+b0+nb] = per-256-block sums of tsq (bf16 squares).

    Pairwise bf16 add levels (DVE 2x mode, ~0.52 ns/out-elem) down to 64
    values per block, then one grouped k=64 f32 reduce (~1.19 ns/in-elem).
    Cheaper than a single k=256 reduce for nb>=4; tiny chunks and the f32
    gpsimd-squared chunk keep the single reduce (per-op overhead isn't
    worth it).
    """
    out = ext[:, 2 + b0 : 2 + b0 + nb]
    if is_f32 or nb <= 2:
        nc.vector.tensor_reduce(
            out=out,
            in_=tsq[:, :].rearrange("p (g k) -> p g k", k=HOP),
            axis=AX.X,
            op=ADD,
        )
        return
    cur, k = tsq, HOP
    while k > 64:
        half = k // 2
        u = casc_pool.tile([P, nb * half], BF16, tag=f"u{k}_{nb}")
        rin = cur[:, :].rearrange("p (g two k) -> p g two k", two=2, k=half)
        rout = u[:, :].rearrange("p (g one k) -> p g one k", one=1, k=half)
        nc.vector.tensor_add(
            out=rout, in0=rin[:, :, 0:1, :], in1=rin[:, :, 1:2, :]
        )
        cur, k = u, half
    nc.vector.tensor_reduce(
        out=out,
        in_=cur[:, :].rearrange("p (g k) -> p g k", k=k),
        axis=AX.X,
        op=ADD,
    )


def build_bass():
    # Bacc (not plain Bass): its compile pipeline splits multi-sem waits into
    # event-semaphore instructions, which this walrus build requires.
    #
    # Bass.__init__ ends with const-AP memsets + an all-engine barrier. The
    # memsets are non-overhead instructions that would open the profiler's
    # exec window ~0.8us before the first load trigger, so suppress both
    # (this kernel reads no const APs: activations get an explicit
    # gpsimd-memset zeros tile that Tile orders itself).
    orig_barrier = bass.Bass.all_engine_barrier
    orig_memset = bass.BassGpSimd.memset
    bass.Bass.all_engine_barrier = lambda self, *, sem_only=False: None
    bass.BassGpSimd.memset = lambda self, ap, constant: None
    try:
        nc = bacc.Bacc()
    finally:
        bass.Bass.all_engine_barrier = orig_barrier
        bass.BassGpSimd.memset = orig_memset
    x = nc.dram_tensor("signal", [SIG_PER_CORE, T], F32, kind="ExternalInput")
    sh = nc.dram_tensor("shifts", [P, 2 * P], F32, kind="ExternalInput")
    y = nc.dram_tensor("out", [SIG_PER_CORE, NFRAMES], F32, kind="ExternalOutput")

    xr = x[:, :].rearrange("b (p f) -> b p f", p=P)   # [2, 128, 8192]
    yr = y[:, :].rearrange("b (p c) -> p b c", p=P)   # [128, 2, 32]

    with SlimExitTileContext(nc) as tc:
        with (
            tc.tile_pool(name="inp", bufs=2) as inp_pool,
            tc.tile_pool(name="sq", bufs=6) as sq_pool,
            tc.tile_pool(name="casc", bufs=2) as casc_pool,
            tc.tile_pool(name="ext", bufs=2) as ext_pool,
            tc.tile_pool(name="small", bufs=2) as small_pool,
            tc.tile_pool(name="ps", bufs=2, space=bass.MemorySpace.PSUM) as psum_pool,
        ):
            # Phase A: enqueue the WHOLE bulk stream on the sync ring first -
            # nothing else ever rides this ring until the output store, so
            # it is never head-of-line blocked. Keyed by block offset since
            # the two signals stream their chunks in different orders.
            tins = [dict() for _ in range(SIG_PER_CORE)]
            shm = small_pool.tile([P, 2 * P], F32, tag="shm")
            n_emitted = 0
            for sig in range(SIG_PER_CORE):
                for b0, nb in SIG_CHUNKS[sig]:
                    ln = nb * HOP
                    tin = inp_pool.tile([P, ln], F32, tag=f"tin{b0}_{nb}")
                    nc.sync.dma_start(
                        out=tin[:, :],
                        in_=xr[sig, :, b0 * HOP : b0 * HOP + ln],
                    )
                    tins[sig][b0] = tin
                    n_emitted += 1
                    if n_emitted == 1:
                        # Shift matrices for the PE seam matmuls (cols
                        # 0:128 = down-shift, 128:256 = up-shift), on the
                        # scalar ring; lands ~9.5us, first matmul needs it
                        # ~13us. Emitted here (2nd in DMA order) so its
                        # completion-sem lane is recycled by a late sig1
                        # chunk with no stall. Seam copies as DMAs are
                        # poison in every variant measured: SWDGE transfers
                        # slow concurrent HWDGE bulk ~40%, and sync-ring
                        # descriptors interleave into the bulk FIFO. The
                        # idle PE does them for free.
                        nc.scalar.dma_start(out=shm[:, :], in_=sh[:, :])

            # Activation-bias zeros via gpsimd memset (no DMA, no const
            # AP). Ready ~6.8us, first consumer needs it ~10.9us.
            zb = small_pool.tile([P, 1], F32, tag="zb")
            nc.gpsimd.memset(zb[:, :], 0.0)

            # Dummy Sqrt first: pulls both ACT table loads (2x 1.28us) into
            # the stream ramp instead of the critical tail. Reads zb so it
            # stays gated.
            dummy = small_pool.tile([1, 1], F32, tag="dummy")
            nc.scalar.activation(
                out=dummy[0:1, 0:1], in_=zb[0:1, 0:1], func=AF.Sqrt,
                bias=zb[0:1, 0:1],
            )

            exts = []
            scrs = []
            pss = []
            for sig in range(SIG_PER_CORE):
                ext = ext_pool.tile([P, 36], F32, tag="ext")
                scr = small_pool.tile([P, 8], F32, tag="scr")
                exts.append(ext)
                scrs.append(scr)

            # e2/ot: both signals' window sums / outputs side by side so one
            # sqrt and one store DMA cover everything.
            e2 = small_pool.tile([P, 2 * CPB], F32, tag="e2")
            ot = small_pool.tile([P, 2 * CPB], F32, tag="ot")

            # Phase B: per signal, per chunk: square (ACT, bf16 out; one
            # tail chunk of sig1 on gpsimd in f32), DVE block reduce; gpsimd
            # edge corrections and PE seam matmuls hang off the first two
            # chunks; seam copies + window sums on DVE after the last
            # reduce of each signal.
            for sig in range(SIG_PER_CORE):
                ext = exts[sig]
                scr = scrs[sig]
                # Compute-emission order: the gpsimd-reduced chunk LAST, so
                # its DVE k-reduce (gated on the gpsimd tree, ~32.5us) does
                # not head-of-line block the micro chunks' reduces (inputs
                # ready ~31.5us) in DVE's in-order queue. Stream order is
                # unchanged.
                order = [c for c in SIG_CHUNKS[sig] if (sig, c[0]) not in GPSIMD_SQ]
                order += [c for c in SIG_CHUNKS[sig] if (sig, c[0]) in GPSIMD_SQ]
                for b0, nb in order:
                    ln = nb * HOP
                    tin = tins[sig][b0]
                    if (sig, b0) in GPSIMD_SQ:
                        tsq = sq_pool.tile([P, ln], F32, tag=f"tsqg{b0}")
                        nc.gpsimd.tensor_mul(
                            out=tsq[:, :], in0=tin[:, :], in1=tin[:, :]
                        )
                        # gpsimd also runs the first 2 pairwise-add levels
                        # (256->64 per block, f32) itself: it is idle after
                        # this square while DVE's queue is the congested
                        # tail resource, so only a ~280ns k=64 reduce lands
                        # on DVE instead of a 679ns k=256 one - and the
                        # gpsimd handoff arrives ~1us sooner than a 3-level
                        # tree would.
                        cur, k = tsq, HOP
                        for _ in range(2):
                            half = k // 2
                            u = casc_pool.tile(
                                [P, nb * half], F32, tag=f"g{k}_{b0}"
                            )
                            rin = cur[:, :].rearrange(
                                "p (g two k) -> p g two k", two=2, k=half
                            )
                            rout = u[:, :].rearrange(
                                "p (g one k) -> p g one k", one=1, k=half
                            )
                            nc.gpsimd.tensor_add(
                                out=rout,
                                in0=rin[:, :, 0:1, :],
                                in1=rin[:, :, 1:2, :],
                            )
                            cur, k = u, half
                        nc.vector.tensor_reduce(
                            out=ext[:, 2 + b0 : 2 + b0 + nb],
                            in_=cur[:, :].rearrange("p (g k) -> p g k", k=k),
                            axis=AX.X,
                            op=ADD,
                        )
                    else:
                        tsq = sq_pool.tile([P, ln], BF16, tag="tsq")
                        nc.scalar.activation(
                            out=tsq[:, :], in_=tin[:, :], func=AF.Square,
                            bias=zb[:, 0:1],
                        )
                        _block_reduce(nc, casc_pool, ext, tsq, b0, nb)
                    if b0 == 28:
                        # Seam 1 matmul: psum[n, 0:2] = ext[n-1, 32:34]
                        # (blocks 30,31 from this chunk's reduce).
                        ps = psum_pool.tile([P, 4], F32, tag=f"ps{sig}")
                        pss.append(ps)
                        nc.tensor.matmul(
                            ps[:, 0:2], shm[:, 0:P], ext[:, 32:34],
                            start=True, stop=True,
                        )
                        # Right reflect edge by correction:
                        #   s_pad[4098] = sum x[T-257:T-1]^2
                        #              = S[4095] + x[T-257]^2 - x[T-1]^2
                        # S[4095] = ext[127, 33] (this chunk's reduce);
                        # x[T-257], x[T-1] are cols 767, 1023 of partition
                        # 127's (28,4) row. Compute partition bases must be
                        # 32-aligned, so run over the 96:128 quadrant; the
                        # garbage written to ext[96:127, 34] is overwritten
                        # by seam copy 2 (ordered after, same engine).
                        v2 = tin[96:128, 767:1024:256]           # [32, 2]
                        nc.gpsimd.tensor_mul(
                            out=scr[96:128, 0:2], in0=v2, in1=v2
                        )
                        nc.gpsimd.tensor_add(
                            out=scr[96:128, 2:3],
                            in0=ext[96:128, 33:34], in1=scr[96:128, 0:1],
                        )
                        nc.gpsimd.tensor_sub(
                            out=ext[96:128, 34:35],
                            in0=scr[96:128, 2:3], in1=scr[96:128, 1:2],
                        )
                    elif b0 == 0:
                        ps = pss[sig]
                        # Seam 2 matmul: psum[n, 2] = ext[n+1, 2] (block 0
                        # from this chunk's reduce). Runs on the idle PE.
                        nc.tensor.matmul(
                            ps[:, 2:3], shm[:, P : 2 * P], ext[:, 2:3],
                            start=True, stop=True,
                        )
                        # Seam copies PSUM -> ext on DVE, right here (mid-
                        # stream) so only the window sums remain in the
                        # post-stream tail. Row 0 of the down-shift and row
                        # 127 of the up-shift are zeros; the left
                        # corrections below overwrite ext[0, 0:2], and the
                        # split copy leaves ext[127, 34] (the right-edge
                        # value written at b0==28) intact.
                        nc.vector.tensor_copy(out=ext[:, 0:2], in_=ps[:, 0:2])
                        nc.vector.tensor_copy(
                            out=ext[0:96, 34:35], in_=ps[0:96, 2:3]
                        )
                        nc.vector.tensor_copy(
                            out=ext[96:127, 34:35], in_=ps[96:127, 2:3]
                        )
                        # Left reflect edges by correction (gpsimd; after
                        # copy1 so they overwrite the zero rows it writes at
                        # partition 0):
                        #   s_pad[1] = S[0] - x[0]^2 + x[256]^2
                        #   s_pad[0] = S[1] - x[256]^2 + x[512]^2
                        # S[0], S[1] = ext[0, 2:4]; x[0], x[256], x[512] are
                        # cols 0,256,512 of partition 0's (0,4) row.
                        v3 = tin[0:1, 0:513:256]                 # [1, 3]
                        nc.gpsimd.tensor_mul(
                            out=scr[0:1, 4:7], in0=v3, in1=v3
                        )
                        nc.gpsimd.tensor_sub(
                            out=scr[0:1, 2:4],
                            in0=ext[0:1, 2:4], in1=scr[0:1, 4:6],
                        )
                        nc.gpsimd.tensor_add(
                            out=ext[0:1, 1:2],
                            in0=scr[0:1, 2:3], in1=scr[0:1, 5:6],
                        )
                        nc.gpsimd.tensor_add(
                            out=ext[0:1, 0:1],
                            in0=scr[0:1, 3:4], in1=scr[0:1, 6:7],
                        )

                # Window-of-4 sums on DVE, right behind this signal's last
                # reduce (sig0's run in the gap before sig1's data-gated
                # reduces; sig1's are the critical tail, with no cross-
                # engine handoff before the final sqrt).
                # E[p, c] = ext[p, c] + ... + ext[p, c+3], via pairwise sums:
                # P1[c] = ext[c] + ext[c+1]; E[c] = P1[c] + P1[c+2].
                p1 = small_pool.tile([P, 34], F32, tag="p1")
                nc.vector.tensor_add(out=p1[:, :], in0=ext[:, 0:34], in1=ext[:, 1:35])
                nc.vector.tensor_add(
                    out=e2[:, sig * CPB : (sig + 1) * CPB],
                    in0=p1[:, 0:32], in1=p1[:, 2:34],
                )

            # One sqrt for both signals, one store DMA for both outputs.
            nc.scalar.activation(
                out=ot[:, :], in_=e2[:, :], func=AF.Sqrt,
                scale=1.0 / N_FFT, bias=zb[:, 0:1],
            )
            nc.sync.dma_start(
                out=yr,
                in_=ot[:, :].rearrange("p (b c) -> p b c", b=SIG_PER_CORE),
            )

    nc.finalize()
    return nc


_NC = None


def _make_shifts() -> np.ndarray:
    m = np.zeros((P, 2 * P), dtype=np.float32)
    # down-shift: psum[n] = mov[n-1]  ->  lhsT[k, n] = 1 iff n == k+1
    m[np.arange(P - 1), np.arange(1, P)] = 1.0
    # up-shift: psum[n] = mov[n+1]   ->  lhsT[k, n] = 1 iff n == k-1
    m[np.arange(1, P), P + np.arange(P - 1)] = 1.0
    return m


_SHIFTS = _make_shifts()


def run(signal: np.ndarray, trace: bool = False):
    global _NC
    sig = np.ascontiguousarray(np.asarray(signal, dtype=np.float32))
    assert sig.shape == (B, T), sig.shape
    if _NC is None:
        _NC = build_bass()
    in_maps = [
        {
            "signal": np.ascontiguousarray(
                sig[k * SIG_PER_CORE : (k + 1) * SIG_PER_CORE]
            ),
            "shifts": _SHIFTS,
        }
        for k in range(N_CORES)
    ]
    try:
        res = run_bass_kernel_spmd(
            _NC, in_maps, core_ids=list(range(N_CORES)), trace=trace
        )
    except Exception:
        # One retry: the shared trn2 devices occasionally surface a
        # transient NRT_EXEC_UNIT_UNRECOVERABLE from a prior session's
        # state; a fresh dispatch typically succeeds.
        res = run_bass_kernel_spmd(
            _NC, in_maps, core_ids=list(range(N_CORES)), trace=trace
        )
    out = np.concatenate([r["out"] for r in res.results], axis=0)
    return out.reshape(B, NFRAMES, 1).astype(np.float32), res


def kernel(signal: np.ndarray) -> np.ndarray:
    out, _ = run(signal, trace=False)
    return out
